# revision 23
# baseline (speedup 1.0000x reference)
"""Trainium2 Bass kernel for per-head causal attention (nn_Attention_52896817217709).

Sharding: 8 cores = 4 head-groups (3 heads each) x 2 batches.
Per core, per head h (S=2048, D_MODEL=768, D_HEAD=64):
  q&k projected together per 512-chunk, packed on the two PE column halves
  (tile_position (0,0)/(0,64)) -> qkT [128,S] (q rows 0:64, k rows 64:128),
  one full-lane DVE evac per chunk; swap-dup into kqT via SBUF->SBUF DMA so
  the scores matmuls can be 2-way row-packed (K=64 halves, concurrent).
  v self-paired on chunk pairs -> checkerboarded vT; vp (PV lhsT [k,d]) via
  XBAR DMA-transpose into offset-0 pool slots (split sync/scalar rings).
  Causal diag-tile masking is an additive PE matmul (identity^T @ maskU).
  Attention runs in two passes over chunk pairs (q 0:1024 then 1024:2048),
  [128,1024] two-bank PSUM score tiles -> one exp per k-tile (scalar engine
  does only exp).  PV z' [64,512] per chunk plus a concurrent col-packed
  M=1 ones-matmul accumulating softmax sums into z row 64.
  out = (z'^T_j @ [W_O; b_O/H]) * rc_j with rc = 1/sums; evac on DVE; fp16 out.
  xq/xk and W_Q/W_K optionally fp8e4m3 (W scaled x16, absorbed in exp scale).
"""
import sys
import os
import numpy as np

for _p in ("/opt/trn_rl_repo", "/root/.axon_site/_ro/trn_rl_repo"):
    if os.path.isdir(_p) and _p not in sys.path:
        sys.path.insert(0, _p)

import ml_dtypes
import concourse.bass as bass
import concourse.tile as tile
from concourse import bacc, mybir
from concourse.bass_utils import run_bass_kernel_spmd

F32 = mybir.dt.float32
FP16 = mybir.dt.float16
FP8 = mybir.dt.float8e4
AF = mybir.ActivationFunctionType

B, S, H, DM, DH = 2, 2048, 12, 768, 64
HPC = 3            # heads per core
NT = S // 128      # 16 k-tiles
MT = DM // 128     # 6 m-tiles
N_CORES = 8
NEG = -60000.0     # additive causal-mask constant (fp16-safe)

USE_FP8 = True     # xq/xk + W_Q/W_K in fp8e4m3 (x16 weight scale)
WSC = 16.0 if USE_FP8 else 1.0
SCALE = 0.125 / (WSC * WSC)   # exp scale absorbs 1/sqrt(DH) and fp8 scaling
XQK_DT = FP8 if USE_FP8 else FP16
NP_X = ml_dtypes.float8_e4m3fn if USE_FP8 else np.float16


def build_program(debug=False):
    nc = bacc.Bacc("TRN2", target_bir_lowering=False, debug=False)

    xq = nc.dram_tensor("xq", [HPC, 2, 128, 3, 2, 1024], XQK_DT,
                        kind="ExternalInput")
    xk = nc.dram_tensor("xk", [HPC, 2, 128, 3, 2, 1024], XQK_DT,
                        kind="ExternalInput")
    xv = nc.dram_tensor("xv", [HPC, 2, 128, MT, 1024], FP16,
                        kind="ExternalInput")
    wp8 = nc.dram_tensor("wp8", [128, HPC * 768], XQK_DT,
                         kind="ExternalInput")
    wp16 = nc.dram_tensor("wp16", [128, HPC * 1152], FP16,
                          kind="ExternalInput")
    bp = nc.dram_tensor("bp", [128, HPC * 3], F32, kind="ExternalInput")
    identh = nc.dram_tensor("identh", [128, 128], FP16, kind="ExternalInput")
    masku = nc.dram_tensor("masku", [128, 128], FP16, kind="ExternalInput")
    out = nc.dram_tensor("out", [HPC, S, DM], FP16, kind="ExternalOutput")
    qscr = nc.dram_tensor("qscr", [HPC, DH, S], FP16, kind="Internal")
    kscr = nc.dram_tensor("kscr", [HPC, DH, S], FP16, kind="Internal")
    zscr = nc.dram_tensor("zscr", [HPC, DH, S], FP16, kind="Internal")
    if debug:
        dqT = nc.dram_tensor("dqT", [128, S], FP16, kind="ExternalOutput")
        dkT = nc.dram_tensor("dkT", [128, S], FP16, kind="ExternalOutput")
        dvT = nc.dram_tensor("dvT", [128, S], FP16, kind="ExternalOutput")
        dvp = nc.dram_tensor("dvp", [128, NT, DH], FP16, kind="ExternalOutput")
        dzT = nc.dram_tensor("dzT", [128, S], FP16, kind="ExternalOutput")
        drc = nc.dram_tensor("drc", [128, NT], F32, kind="ExternalOutput")

    with tile.TileContext(nc) as tc:
        with (
            tc.tile_pool(name="wpool", bufs=1) as wpool,
            tc.tile_pool(name="xp", bufs=4) as x_pool,
            tc.tile_pool(name="wt", bufs=2) as wt_pool,
            tc.tile_pool(name="qk", bufs=2) as qk_pool,
            tc.tile_pool(name="vp", bufs=24) as vp_pool,
            tc.tile_pool(name="pp", bufs=6) as p_pool,
            tc.tile_pool(name="zt", bufs=2) as zt_pool,
            tc.tile_pool(name="rc", bufs=2) as rc_pool,
            tc.tile_pool(name="ob", bufs=3) as ob_pool,
            tc.tile_pool(name="psa", bufs=2, space="PSUM") as ps_aux,
            tc.tile_pool(name="pss", bufs=2, space="PSUM") as ps_s,
            tc.tile_pool(name="psz", bufs=2, space="PSUM") as ps_z,
        ):
            id_sb = wpool.tile([128, 128], FP16, name="id_sb")
            nc.gpsimd.dma_start(id_sb[:], identh[:])
            mask_sb = wpool.tile([128, 128], FP16, name="mask_sb")
            nc.gpsimd.dma_start(mask_sb[:], masku[:])
            ones_sb = wpool.tile([128, 1], FP16, name="ones_sb")
            nc.gpsimd.memset(ones_sb[:], 1.0)
            w8_sb = wpool.tile([128, HPC * 768], XQK_DT, name="w8_sb")
            nc.sync.dma_start(w8_sb[:], wp8[:])
            w16_sb = wpool.tile([128, HPC * 1152], FP16, name="w16_sb")
            nc.sync.dma_start(w16_sb[:], wp16[:])
            bp_sb = wpool.tile([128, HPC * 3], F32, name="bp_sb")
            nc.sync.dma_start(bp_sb[:], bp[:])

            st = [dict() for _ in range(HPC)]
            for _h in range(HPC):
                st[_h]["wq"] = w8_sb[:, 768 * _h:768 * _h + 384].rearrange(
                    "p (b k d) -> p b k d", b=3, k=2)
                st[_h]["wk"] = w8_sb[:, 768 * _h + 384:768 * _h + 768
                                     ].rearrange("p (b k d) -> p b k d",
                                                 b=3, k=2)
                st[_h]["wv"] = w16_sb[:, 1152 * _h:1152 * _h + 384
                                      ].rearrange("p (a d) -> p a d", a=MT)
                st[_h]["wo"] = w16_sb[:, 1152 * _h + 384:1152 * _h + 1152]
                st[_h]["bq"] = bp_sb[:, 3 * _h:3 * _h + 1]
                st[_h]["bk"] = bp_sb[:, 3 * _h + 1:3 * _h + 2]
                st[_h]["bv"] = bp_sb[:, 3 * _h + 2:3 * _h + 3]

            def emit_loads(h):
                """xq/xk halves on sync; xv on gpsimd."""
                for t, xd in (("q", xq), ("k", xk)):
                    halves = []
                    for a in range(2):
                        xt = x_pool.tile([128, 3, 2, 1024], XQK_DT,
                                         name=f"x{t}{h}{a}", tag=f"x{t}")
                        nc.sync.dma_start(xt[:], xd[h, a])
                        halves.append(xt)
                    st[h][f"x{t}"] = halves
                halves = []
                for a in range(2):
                    xt = x_pool.tile([128, MT, 1024], FP16,
                                     name=f"xv{h}{a}", tag="xv")
                    nc.gpsimd.dma_start(xt[:], xv[h, a])
                    halves.append(xt)
                st[h]["xv"] = halves

            def emit_proj_qk(h):
                """q,k DoubleRow projections -> qT/kT, dup'd to both halves."""
                qT = qk_pool.tile([128, S], FP16, name=f"qT{h}", tag="qT")
                kT = qk_pool.tile([128, S], FP16, name=f"kT{h}", tag="kT")
                st[h]["qT"], st[h]["kT"] = qT, kT
                DR = mybir.MatmulPerfMode.DoubleRow
                for c in range(4):
                    off = (c % 2) * 512
                    for t, dst in (("q", qT), ("k", kT)):
                        xt = st[h][f"x{t}"][c // 2]
                        wt = st[h][f"w{t}"]
                        b = st[h][f"b{t}"]
                        acc = ps_aux.tile([128, 512], F32,
                                          name=f"a{t}{h}{c}", tag="a")
                        for bb in range(3):
                            nc.tensor.matmul(
                                acc[0:DH, :], wt[:, bb, :, :],
                                xt[:, bb, :, off:off + 512],
                                start=(bb == 0), stop=(bb == 2),
                                perf_mode=DR)
                        nc.vector.tensor_scalar_add(
                            dst[0:DH, bass.ts(c, 512)], acc[0:DH, :],
                            b[0:DH])
                        scr = qscr if t == "q" else kscr
                        ring = nc.sync if t == "q" else nc.gpsimd
                        ring.dma_start(scr[h][:, bass.ts(c, 512)],
                                       dst[0:DH, bass.ts(c, 512)])
                        ring.dma_start(dst[DH:128, bass.ts(c, 512)],
                                       scr[h][:, bass.ts(c, 512)])

            def emit_proj_v(h):
                """v self-paired on chunk pairs -> checkerboarded vT."""
                vT = qk_pool.tile([128, S], FP16, name=f"vT{h}", tag="vT")
                st[h]["vT"] = vT
                w, b = st[h]["wv"], st[h]["bv"]
                for pr in range(2):
                    xt = st[h]["xv"][pr]
                    acc = ps_aux.tile([128, 512], F32, name=f"av{h}{pr}",
                                      tag="a")
                    for mt in range(MT):
                        nc.tensor.matmul(
                            acc[0:DH, :], w[:, mt, :], xt[:, mt, 0:512],
                            start=(mt == 0), stop=(mt == MT - 1),
                            tile_position=(0, 0))
                        nc.tensor.matmul(
                            acc[DH:128, :], w[:, mt, :], xt[:, mt, 512:1024],
                            start=(mt == 0), stop=(mt == MT - 1),
                            tile_position=(0, DH))
                    c0, c1 = 2 * pr, 2 * pr + 1
                    nc.vector.tensor_scalar_add(
                        vT[0:DH, bass.ts(c0, 512)], acc[0:DH, :], b[0:DH])
                    nc.vector.tensor_scalar_add(
                        vT[DH:128, bass.ts(c1, 512)], acc[DH:128, :],
                        b[DH:128])

            def emit_vp(h):
                """PV lhsT [k, d|1] per k-tile via PE transpose + DVE copy."""
                vT = st[h]["vT"]
                vps = []
                for i in range(NT):
                    r0 = 0 if (i // 4) % 2 == 0 else DH
                    vt = vp_pool.tile([128, DH + 4], FP16, name=f"vp{h}_{i}",
                                      tag="vp")
                    v_ps = ps_aux.tile([128, DH], FP16, name=f"vps{h}{i}",
                                       tag="a", padded_shape=[128, 1024])
                    nc.tensor.transpose(v_ps[:], vT[r0:r0 + DH, bass.ts(i, 128)],
                                        id_sb[r0:r0 + DH, r0:r0 + DH])
                    nc.vector.tensor_copy(vt[:, 0:DH], v_ps[:])
                    nc.gpsimd.memset(vt[:, DH:DH + 1], 1.0)
                    vps.append(vt)
                st[h]["vp"] = vps

            def stage_pair(h, i0, qhi):
                """Scores + exp for k-tiles i0, i0+1 (row-packed halves)."""
                qT, kT = st[h]["qT"], st[h]["kT"]
                res = []
                for i, pos in ((i0, 0), (i0 + 1, DH)):
                    qlo = max(128 * i, qhi - 1024)
                    w = qhi - qlo
                    sp = ps_s.tile([128, 1024], F32, name=f"s{h}{i}{qhi}",
                                   tag="s")
                    diag = qlo == 128 * i
                    kt = kT[pos:pos + DH, bass.ts(i, 128)]
                    qt = qT
                    for o in range(0, w, 512):
                        ww = min(512, w - o)
                        nc.tensor.matmul(sp[:, o:o + ww], kt,
                                         qt[pos:pos + DH,
                                            qlo + o:qlo + o + ww],
                                         start=True,
                                         stop=not (diag and o == 0))
                        if diag and o == 0:
                            nc.tensor.matmul(sp[:, 0:128], id_sb[:],
                                             mask_sb[:], start=False,
                                             stop=True)
                    P = p_pool.tile([128, 1024], FP16, name=f"P{h}{i}{qhi}",
                                    tag="P")
                    nc.scalar.activation(P[:, 0:w], sp[:, 0:w], AF.Exp,
                                         scale=SCALE)
                    res.append((P, qlo))
                return res

            def finish_chunk(h, c, zps):
                zT, rc, srow = st[h]["zT"], st[h]["rc"], st[h]["srow"]
                nc.vector.tensor_copy(zT[0:DH, bass.ts(c, 512)], zps[0:DH, :])
                nc.vector.tensor_copy(srow[DH:DH + 1, bass.ts(c, 512)],
                                      zps[DH:DH + 1, :])
                rcp = ps_aux.tile([128, 8], FP16, name=f"rcp{h}{c}", tag="a",
                                  padded_shape=[128, 1024])
                for j in range(4):
                    nc.tensor.transpose(
                        rcp[:, 2 * j:2 * j + 1],
                        srow[DH:DH + 1,
                             512 * c + 128 * j:512 * c + 128 * j + 128],
                        id_sb[DH:DH + 1, DH:DH + 1])
                nc.vector.reciprocal(rc[:, 4 * c:4 * c + 4], rcp[:, 0:8:2])
                nc.gpsimd.dma_start(zscr[h][:, bass.ts(c, 512)],
                                    zT[0:DH, bass.ts(c, 512)])
                nc.gpsimd.dma_start(zT[DH:128, bass.ts(c, 512)],
                                    zscr[h][:, bass.ts(c, 512)])

            def emit_pass(h, cpair, hooks):
                """Attention pass over chunks cpair=(c0,c1); i-major PVs."""
                c0, c1 = cpair
                qhi = 512 * c1 + 512
                nk = 4 * c1 + 4
                vp = st[h]["vp"]
                if c0 == 0:
                    zT = zt_pool.tile([128, S], FP16, name=f"zT{h}", tag="zT")
                    rc = rc_pool.tile([128, NT], F32, name=f"rc{h}", tag="rc")
                    srow = rc_pool.tile([DH + 1, S], FP16, name=f"srow{h}",
                                        tag="srow")
                    st[h]["srow"] = srow
                    st[h]["zT"] = zT
                    st[h]["rc"] = rc
                z0 = ps_z.tile([DH + 1, 512], F32, name=f"z{h}{c0}", tag="z")
                z1 = ps_z.tile([DH + 1, 512], F32, name=f"z{h}{c1}", tag="z")
                staged = {}
                for i0 in (0, 2):
                    for P, j in zip(stage_pair(h, i0, qhi), (i0, i0 + 1)):
                        staged[j] = P
                for i in range(nk):
                    if i % 2 == 0 and i + 4 < nk:
                        for P, j in zip(stage_pair(h, i + 4, qhi),
                                        (i + 4, i + 5)):
                            staged[j] = P
                    P, qlo = staged[i]
                    for c, z in ((c0, z0), (c1, z1)):
                        if i >= 4 * c + 4:
                            continue
                        ql = max(512 * c, 128 * i)
                        w = 512 * c + 512 - ql
                        zc = ql - 512 * c
                        Pc = P[:, ql - qlo:ql - qlo + w]
                        nc.tensor.matmul(
                            z[:, zc:zc + w], vp[i][:, 0:DH + 1], Pc,
                            start=(i == 0), stop=(i == 4 * c + 3))
                    del staged[i]
                    if i == 4 * c0 + 3:
                        finish_chunk(h, c0, z0)
                        for f in hooks.get(c0, []):
                            f()
                finish_chunk(h, c1, z1)
                for f in hooks.get(c1, []):
                    f()

            def emit_outproj(h, jjs):
                zT, rc, wot = st[h]["zT"], st[h]["rc"], st[h]["wo"]
                for jj in jjs:
                    ob = ob_pool.tile([128, 2, DM], FP16, name=f"ob{h}{jj}",
                                      tag="ob")
                    j0, j1 = 2 * jj, 2 * jj + 1
                    for mo, mw in ((0, 512), (512, 256)):
                        apsA = ps_aux.tile([128, 512], F32,
                                           name=f"o{h}{j0}{mo}", tag="a")
                        apsB = ps_aux.tile([128, 512], F32,
                                           name=f"o{h}{j1}{mo}", tag="a")
                        nc.tensor.matmul(apsA[:, 0:mw],
                                         zT[0:DH, bass.ts(j0, 128)],
                                         wot[0:DH, mo:mo + mw],
                                         start=True, stop=True)
                        nc.tensor.matmul(apsB[:, 0:mw],
                                         zT[DH:128, bass.ts(j1, 128)],
                                         wot[DH:128, mo:mo + mw],
                                         start=True, stop=True)
                        if mo == 0:
                            nc.vector.tensor_scalar_mul(
                                ob[:, 0, mo:mo + mw], apsA[:, 0:mw],
                                rc[:, j0:j0 + 1])
                            nc.vector.tensor_scalar_mul(
                                ob[:, 1, mo:mo + mw], apsB[:, 0:mw],
                                rc[:, j1:j1 + 1])
                        else:
                            nc.scalar.activation(
                                ob[:, 0, mo:mo + mw], apsA[:, 0:mw],
                                AF.Copy, scale=rc[:, j0:j0 + 1])
                            nc.scalar.activation(
                                ob[:, 1, mo:mo + mw], apsB[:, 0:mw],
                                AF.Copy, scale=rc[:, j1:j1 + 1])
                    ring = nc.gpsimd if jj % 2 == 0 else nc.sync
                    ring.dma_start(
                        out[h, bass.ts(jj, 256), :]
                           .rearrange("(a p) m -> p a m", p=128),
                        ob[:])

            emit_loads(0)
            emit_proj_qk(0)
            emit_proj_v(0)
            emit_vp(0)
            for h in range(HPC):
                nxt, prv = h + 1, h - 1
                if nxt < HPC:
                    emit_loads(nxt)
                acts = {0: [], 1: [], 2: [], 3: []}
                if prv >= 0:
                    acts[0].append(lambda p=prv: emit_outproj(p, (4, 5)))
                    acts[1].append(lambda p=prv: emit_outproj(p, (6, 7)))
                if nxt < HPC:
                    acts[1].append(lambda n=nxt: emit_proj_qk(n))
                    acts[2].append(lambda n=nxt: emit_proj_v(n))
                    acts[3].append(lambda n=nxt: emit_vp(n))
                if debug and h == 0:
                    nc.gpsimd.dma_start(dqT[:], st[0]["qT"][:])
                    nc.gpsimd.dma_start(dkT[:], st[0]["kT"][:])
                    nc.gpsimd.dma_start(dvT[:], st[0]["vT"][:])
                    for i in range(NT):
                        nc.gpsimd.dma_start(dvp[:, i, :], st[0]["vp"][i][:, 0:DH])
                emit_pass(h, (0, 1), {c: acts[c] for c in (0, 1)})
                emit_outproj(h, (0, 1, 2, 3))
                emit_pass(h, (2, 3), {c: acts[c] for c in (2, 3)})
                if debug and h == 0:
                    nc.gpsimd.dma_start(dzT[:], st[0]["zT"][:])
                    nc.gpsimd.dma_start(drc[:], st[0]["rc"][:])
            emit_outproj(HPC - 1, (4, 5, 6, 7))
    nc.compile()
    return nc


_CACHED = None


def _program(debug=False):
    global _CACHED
    if _CACHED is None:
        _CACHED = build_program(debug)
    return _CACHED


def _make_in_maps(inputs):
    xq_f = np.asarray(inputs["normalized_resid_pre_q"], dtype=np.float32)
    xk_f = np.asarray(inputs["normalized_resid_pre_k"], dtype=np.float32)
    xv_f = np.asarray(inputs["normalized_resid_pre_v"], dtype=np.float32)
    WQ = np.asarray(inputs["W_Q"], dtype=np.float32) * WSC
    WK = np.asarray(inputs["W_K"], dtype=np.float32) * WSC
    WV = np.asarray(inputs["W_V"], dtype=np.float32)
    WO = np.asarray(inputs["W_O"], dtype=np.float32)
    bQ = np.asarray(inputs["b_Q"], dtype=np.float32) * WSC
    bK = np.asarray(inputs["b_K"], dtype=np.float32) * WSC
    bV = np.asarray(inputs["b_V"], dtype=np.float32)
    bO = np.asarray(inputs["b_O"], dtype=np.float32)

    def interleave_x(x):  # [DM, S] -> [2, 128, 3, 2, 1024] (s-halves split)
        y = x.reshape(3, 2, 128, 2, 1024)
        return np.ascontiguousarray(y.transpose(3, 2, 0, 1, 4))

    def interleave_w(w):  # [DM, DH] -> [128, 3, 2, DH]
        return np.ascontiguousarray(
            w.reshape(3, 2, 128, DH).transpose(2, 0, 1, 3))

    identh = np.eye(128, dtype=np.float16)
    masku = ((np.arange(128)[:, None] > np.arange(128)[None, :])
             .astype(np.float16) * np.float16(NEG))

    bq2 = np.zeros((H, 128, 1), np.float32)
    bq2[:, 0:DH, 0] = bQ
    bq2[:, DH:128, 0] = bQ
    bk2 = np.zeros((H, 128, 1), np.float32)
    bk2[:, 0:DH, 0] = bK
    bk2[:, DH:128, 0] = bK
    bv2 = np.zeros((H, 128, 1), np.float32)
    bv2[:, 0:DH, 0] = bV
    bv2[:, DH:128, 0] = bV

    in_maps = []
    for c in range(N_CORES):
        b = c % 2
        hg = c // 2
        hs = slice(HPC * hg, HPC * hg + HPC)
        m = {
            "xq": np.stack([interleave_x(
                xq_f[b, :, HPC * hg + j, :].T) for j in range(HPC)]
                ).astype(NP_X),
            "xk": np.stack([interleave_x(
                xk_f[b, :, HPC * hg + j, :].T) for j in range(HPC)]
                ).astype(NP_X),
            "xv": np.ascontiguousarray(
                xv_f[b, :, hs, :].transpose(1, 2, 0)
                .reshape(HPC, MT, 128, 2, 1024).transpose(0, 3, 2, 1, 4)
                ).astype(np.float16),
            "wp8": np.concatenate(sum((
                [interleave_w(WQ[HPC * hg + j]).reshape(128, 384),
                 interleave_w(WK[HPC * hg + j]).reshape(128, 384)]
                for j in range(HPC)), []), axis=1).astype(NP_X),
            "wp16": np.concatenate(sum((
                [WV[HPC * hg + j].reshape(MT, 128, DH)
                 .transpose(1, 0, 2).reshape(128, 384),
                 np.concatenate([WO[HPC * hg + j], WO[HPC * hg + j]],
                                axis=0)]
                for j in range(HPC)), []), axis=1).astype(np.float16),
            "bp": np.concatenate(sum((
                [bq2[HPC * hg + j], bk2[HPC * hg + j], bv2[HPC * hg + j]]
                for j in range(HPC)), []), axis=1).astype(np.float32),
            "identh": identh,
            "masku": masku,
        }
        in_maps.append(m)
    return in_maps


def run(inputs, trace=False, debug=False, **kw):
    nc = _program(debug)
    in_maps = _make_in_maps(inputs)
    res = run_bass_kernel_spmd(nc, in_maps, core_ids=list(range(N_CORES)),
                               trace=trace, **kw)
    full = np.zeros((B, S, H, DM), np.float32)
    for c in range(N_CORES):
        b = c % 2
        hg = c // 2
        o = res.results[c]["out"]
        for j in range(HPC):
            full[b, :, HPC * hg + j, :] = o[j]
    bO = np.asarray(inputs["b_O"], dtype=np.float32)
    if np.any(bO):
        full += bO / H
    return full, res


def kernel(**inputs):
    full, _ = run(inputs)
    return full


# revision 24
# speedup vs baseline: 1.0599x; 1.0599x over previous
"""Trainium2 Bass kernel for per-head causal attention (nn_Attention_52896817217709).

Sharding: 8 cores = 4 head-groups (3 heads each) x 2 batches.
Per core, per head h (S=2048, D_MODEL=768, D_HEAD=64):
  q&k projected together per 512-chunk, packed on the two PE column halves
  (tile_position (0,0)/(0,64)) -> qkT [128,S] (q rows 0:64, k rows 64:128),
  one full-lane DVE evac per chunk; swap-dup into kqT via SBUF->SBUF DMA so
  the scores matmuls can be 2-way row-packed (K=64 halves, concurrent).
  v self-paired on chunk pairs -> checkerboarded vT; vp (PV lhsT [k,d]) via
  XBAR DMA-transpose into offset-0 pool slots (split sync/scalar rings).
  Causal diag-tile masking is an additive PE matmul (identity^T @ maskU).
  Attention runs in two passes over chunk pairs (q 0:1024 then 1024:2048),
  [128,1024] two-bank PSUM score tiles -> one exp per k-tile (scalar engine
  does only exp).  PV z' [64,512] per chunk plus a concurrent col-packed
  M=1 ones-matmul accumulating softmax sums into z row 64.
  out = (z'^T_j @ [W_O; b_O/H]) * rc_j with rc = 1/sums; evac on DVE; fp16 out.
  xq/xk and W_Q/W_K optionally fp8e4m3 (W scaled x16, absorbed in exp scale).
"""
import sys
import os
import numpy as np

for _p in ("/opt/trn_rl_repo", "/root/.axon_site/_ro/trn_rl_repo"):
    if os.path.isdir(_p) and _p not in sys.path:
        sys.path.insert(0, _p)

import ml_dtypes
import concourse.bass as bass
import concourse.tile as tile
from concourse import bacc, mybir
from concourse.bass_utils import run_bass_kernel_spmd

F32 = mybir.dt.float32
FP16 = mybir.dt.float16
FP8 = mybir.dt.float8e4
AF = mybir.ActivationFunctionType

B, S, H, DM, DH = 2, 2048, 12, 768, 64
HPC = 3            # heads per core
NT = S // 128      # 16 k-tiles
MT = DM // 128     # 6 m-tiles
N_CORES = 8
NEG = -60000.0     # additive causal-mask constant (fp16-safe)

USE_FP8 = True     # xq/xk + W_Q/W_K in fp8e4m3 (x16 weight scale)
WSC = 16.0 if USE_FP8 else 1.0
SCALE = 0.125 / (WSC * WSC)   # exp scale absorbs 1/sqrt(DH) and fp8 scaling
XQK_DT = FP8 if USE_FP8 else FP16
NP_X = ml_dtypes.float8_e4m3fn if USE_FP8 else np.float16


def build_program(debug=False):
    nc = bacc.Bacc("TRN2", target_bir_lowering=False, debug=False)

    xq = nc.dram_tensor("xq", [HPC, 2, 128, 3, 2, 1024], XQK_DT,
                        kind="ExternalInput")
    xk = nc.dram_tensor("xk", [HPC, 2, 128, 3, 2, 1024], XQK_DT,
                        kind="ExternalInput")
    xv = nc.dram_tensor("xv", [HPC, 2, 128, MT, 1024], FP16,
                        kind="ExternalInput")
    wp8 = nc.dram_tensor("wp8", [128, HPC * 768], XQK_DT,
                         kind="ExternalInput")
    wp16 = nc.dram_tensor("wp16", [128, HPC * 1152], FP16,
                          kind="ExternalInput")
    bp = nc.dram_tensor("bp", [128, HPC * 3], F32, kind="ExternalInput")
    identh = nc.dram_tensor("identh", [128, 128], FP16, kind="ExternalInput")
    masku = nc.dram_tensor("masku", [128, 128], FP16, kind="ExternalInput")
    out = nc.dram_tensor("out", [HPC, S, DM], FP16, kind="ExternalOutput")
    qscr = nc.dram_tensor("qscr", [HPC, DH, S], FP16, kind="Internal")
    kscr = nc.dram_tensor("kscr", [HPC, DH, S], FP16, kind="Internal")
    zscr = nc.dram_tensor("zscr", [HPC, DH, S], FP16, kind="Internal")
    if debug:
        dqT = nc.dram_tensor("dqT", [128, S], FP16, kind="ExternalOutput")
        dkT = nc.dram_tensor("dkT", [128, S], FP16, kind="ExternalOutput")
        dvT = nc.dram_tensor("dvT", [128, S], FP16, kind="ExternalOutput")
        dvp = nc.dram_tensor("dvp", [128, NT, DH], FP16, kind="ExternalOutput")
        dzT = nc.dram_tensor("dzT", [128, S], FP16, kind="ExternalOutput")
        drc = nc.dram_tensor("drc", [128, NT], F32, kind="ExternalOutput")

    with tile.TileContext(nc) as tc:
        with (
            tc.tile_pool(name="wpool", bufs=1) as wpool,
            tc.tile_pool(name="xp", bufs=4) as x_pool,
            tc.tile_pool(name="wt", bufs=2) as wt_pool,
            tc.tile_pool(name="qk", bufs=2) as qk_pool,
            tc.tile_pool(name="vp", bufs=24) as vp_pool,
            tc.tile_pool(name="pp", bufs=6) as p_pool,
            tc.tile_pool(name="zt", bufs=2) as zt_pool,
            tc.tile_pool(name="rc", bufs=2) as rc_pool,
            tc.tile_pool(name="ob", bufs=3) as ob_pool,
            tc.tile_pool(name="psa", bufs=2, space="PSUM") as ps_aux,
            tc.tile_pool(name="pss", bufs=2, space="PSUM") as ps_s,
            tc.tile_pool(name="psz", bufs=2, space="PSUM") as ps_z,
        ):
            id_sb = wpool.tile([128, 128], FP16, name="id_sb")
            nc.gpsimd.dma_start(id_sb[:], identh[:])
            mask_sb = wpool.tile([128, 128], FP16, name="mask_sb")
            nc.gpsimd.dma_start(mask_sb[:], masku[:])
            ones_sb = wpool.tile([128, 1], FP16, name="ones_sb")
            nc.gpsimd.memset(ones_sb[:], 1.0)
            w8_sb = wpool.tile([128, HPC * 768], XQK_DT, name="w8_sb")
            nc.sync.dma_start(w8_sb[:], wp8[:])
            w16_sb = wpool.tile([128, HPC * 1152], FP16, name="w16_sb")
            nc.sync.dma_start(w16_sb[:], wp16[:])
            bp_sb = wpool.tile([128, HPC * 3], F32, name="bp_sb")
            nc.sync.dma_start(bp_sb[:], bp[:])

            st = [dict() for _ in range(HPC)]
            for _h in range(HPC):
                st[_h]["wq"] = w8_sb[:, 768 * _h:768 * _h + 384].rearrange(
                    "p (b k d) -> p b k d", b=3, k=2)
                st[_h]["wk"] = w8_sb[:, 768 * _h + 384:768 * _h + 768
                                     ].rearrange("p (b k d) -> p b k d",
                                                 b=3, k=2)
                st[_h]["wv"] = w16_sb[:, 1152 * _h:1152 * _h + 384
                                      ].rearrange("p (a d) -> p a d", a=MT)
                st[_h]["wo"] = w16_sb[:, 1152 * _h + 384:1152 * _h + 1152]
                st[_h]["bq"] = bp_sb[:, 3 * _h:3 * _h + 1]
                st[_h]["bk"] = bp_sb[:, 3 * _h + 1:3 * _h + 2]
                st[_h]["bv"] = bp_sb[:, 3 * _h + 2:3 * _h + 3]

            def emit_loads(h):
                """xq/xk halves on sync; xv on gpsimd."""
                for t, xd in (("q", xq), ("k", xk)):
                    halves = []
                    for a in range(2):
                        xt = x_pool.tile([128, 3, 2, 1024], XQK_DT,
                                         name=f"x{t}{h}{a}", tag=f"x{t}")
                        nc.sync.dma_start(xt[:], xd[h, a])
                        halves.append(xt)
                    st[h][f"x{t}"] = halves
                halves = []
                for a in range(2):
                    xt = x_pool.tile([128, MT, 1024], FP16,
                                     name=f"xv{h}{a}", tag="xv")
                    nc.gpsimd.dma_start(xt[:], xv[h, a])
                    halves.append(xt)
                st[h]["xv"] = halves

            def emit_proj_qk(h):
                """q,k DoubleRow projections -> qT/kT, dup'd to both halves."""
                qT = qk_pool.tile([128, S], FP16, name=f"qT{h}", tag="qT")
                kT = qk_pool.tile([128, S], FP16, name=f"kT{h}", tag="kT")
                st[h]["qT"], st[h]["kT"] = qT, kT
                DR = mybir.MatmulPerfMode.DoubleRow
                for c in range(4):
                    off = (c % 2) * 512
                    for t, dst in (("q", qT), ("k", kT)):
                        xt = st[h][f"x{t}"][c // 2]
                        wt = st[h][f"w{t}"]
                        b = st[h][f"b{t}"]
                        acc = ps_aux.tile([128, 512], F32,
                                          name=f"a{t}{h}{c}", tag="a")
                        for bb in range(3):
                            nc.tensor.matmul(
                                acc[0:DH, :], wt[:, bb, :, :],
                                xt[:, bb, :, off:off + 512],
                                start=(bb == 0), stop=(bb == 2),
                                perf_mode=DR)
                        nc.vector.tensor_scalar_add(
                            dst[0:DH, bass.ts(c, 512)], acc[0:DH, :],
                            b[0:DH])
                        ring = nc.sync if t == "q" else nc.gpsimd
                        ring.dma_start(dst[DH:128, bass.ts(c, 512)],
                                       dst[0:DH, bass.ts(c, 512)])

            def emit_proj_v(h):
                """v self-paired on chunk pairs -> checkerboarded vT."""
                vT = qk_pool.tile([128, S], FP16, name=f"vT{h}", tag="vT")
                st[h]["vT"] = vT
                w, b = st[h]["wv"], st[h]["bv"]
                for pr in range(2):
                    xt = st[h]["xv"][pr]
                    acc = ps_aux.tile([128, 512], F32, name=f"av{h}{pr}",
                                      tag="a")
                    for mt in range(MT):
                        nc.tensor.matmul(
                            acc[0:DH, :], w[:, mt, :], xt[:, mt, 0:512],
                            start=(mt == 0), stop=(mt == MT - 1),
                            tile_position=(0, 0))
                        nc.tensor.matmul(
                            acc[DH:128, :], w[:, mt, :], xt[:, mt, 512:1024],
                            start=(mt == 0), stop=(mt == MT - 1),
                            tile_position=(0, DH))
                    c0, c1 = 2 * pr, 2 * pr + 1
                    nc.vector.tensor_scalar_add(
                        vT[0:DH, bass.ts(c0, 512)], acc[0:DH, :], b[0:DH])
                    nc.vector.tensor_scalar_add(
                        vT[DH:128, bass.ts(c1, 512)], acc[DH:128, :],
                        b[DH:128])

            def emit_vp(h):
                """PV lhsT [k, d|1] per k-tile via PE transpose + DVE copy."""
                vT = st[h]["vT"]
                vps = []
                for i in range(NT):
                    r0 = 0 if (i // 4) % 2 == 0 else DH
                    vt = vp_pool.tile([128, DH + 4], FP16, name=f"vp{h}_{i}",
                                      tag="vp")
                    v_ps = ps_aux.tile([128, DH], FP16, name=f"vps{h}{i}",
                                       tag="a", padded_shape=[128, 1024])
                    nc.tensor.transpose(v_ps[:], vT[r0:r0 + DH, bass.ts(i, 128)],
                                        id_sb[r0:r0 + DH, r0:r0 + DH])
                    nc.vector.tensor_copy(vt[:, 0:DH], v_ps[:])
                    nc.gpsimd.memset(vt[:, DH:DH + 1], 1.0)
                    vps.append(vt)
                st[h]["vp"] = vps

            def stage_pair(h, i0, qhi):
                """Scores + exp for k-tiles i0, i0+1 (row-packed halves)."""
                qT, kT = st[h]["qT"], st[h]["kT"]
                res = []
                for i, pos in ((i0, 0), (i0 + 1, DH)):
                    qlo = max(128 * i, qhi - 1024)
                    w = qhi - qlo
                    sp = ps_s.tile([128, 1024], F32, name=f"s{h}{i}{qhi}",
                                   tag="s")
                    diag = qlo == 128 * i
                    kt = kT[pos:pos + DH, bass.ts(i, 128)]
                    qt = qT
                    for o in range(0, w, 512):
                        ww = min(512, w - o)
                        nc.tensor.matmul(sp[:, o:o + ww], kt,
                                         qt[pos:pos + DH,
                                            qlo + o:qlo + o + ww],
                                         start=True,
                                         stop=not (diag and o == 0))
                        if diag and o == 0:
                            nc.tensor.matmul(sp[:, 0:128], id_sb[:],
                                             mask_sb[:], start=False,
                                             stop=True)
                    P = p_pool.tile([128, 1024], FP16, name=f"P{h}{i}{qhi}",
                                    tag="P")
                    nc.scalar.activation(P[:, 0:w], sp[:, 0:w], AF.Exp,
                                         scale=SCALE)
                    res.append((P, qlo))
                return res

            def finish_chunk(h, c, zps):
                zT, rc, srow = st[h]["zT"], st[h]["rc"], st[h]["srow"]
                nc.vector.tensor_copy(zT[0:DH, bass.ts(c, 512)], zps[0:DH, :])
                nc.vector.tensor_copy(srow[DH:DH + 1, bass.ts(c, 512)],
                                      zps[DH:DH + 1, :])
                rcp = ps_aux.tile([128, 8], FP16, name=f"rcp{h}{c}", tag="a",
                                  padded_shape=[128, 1024])
                for j in range(4):
                    nc.tensor.transpose(
                        rcp[:, 2 * j:2 * j + 1],
                        srow[DH:DH + 1,
                             512 * c + 128 * j:512 * c + 128 * j + 128],
                        id_sb[DH:DH + 1, DH:DH + 1])
                nc.vector.reciprocal(rc[:, 4 * c:4 * c + 4], rcp[:, 0:8:2])
                nc.gpsimd.dma_start(zT[DH:128, bass.ts(c, 512)],
                                    zT[0:DH, bass.ts(c, 512)])

            def emit_pass(h, cpair, hooks):
                """Attention pass over chunks cpair=(c0,c1); i-major PVs."""
                c0, c1 = cpair
                qhi = 512 * c1 + 512
                nk = 4 * c1 + 4
                vp = st[h]["vp"]
                if c0 == 0:
                    zT = zt_pool.tile([128, S], FP16, name=f"zT{h}", tag="zT")
                    rc = rc_pool.tile([128, NT], F32, name=f"rc{h}", tag="rc")
                    srow = rc_pool.tile([DH + 1, S], FP16, name=f"srow{h}",
                                        tag="srow")
                    st[h]["srow"] = srow
                    st[h]["zT"] = zT
                    st[h]["rc"] = rc
                z0 = ps_z.tile([DH + 1, 512], F32, name=f"z{h}{c0}", tag="z")
                z1 = ps_z.tile([DH + 1, 512], F32, name=f"z{h}{c1}", tag="z")
                staged = {}
                for i0 in (0, 2):
                    for P, j in zip(stage_pair(h, i0, qhi), (i0, i0 + 1)):
                        staged[j] = P
                for i in range(nk):
                    if i % 2 == 0 and i + 4 < nk:
                        for P, j in zip(stage_pair(h, i + 4, qhi),
                                        (i + 4, i + 5)):
                            staged[j] = P
                    P, qlo = staged[i]
                    for c, z in ((c0, z0), (c1, z1)):
                        if i >= 4 * c + 4:
                            continue
                        ql = max(512 * c, 128 * i)
                        w = 512 * c + 512 - ql
                        zc = ql - 512 * c
                        Pc = P[:, ql - qlo:ql - qlo + w]
                        nc.tensor.matmul(
                            z[:, zc:zc + w], vp[i][:, 0:DH + 1], Pc,
                            start=(i == 0), stop=(i == 4 * c + 3))
                    del staged[i]
                    if i == 4 * c0 + 3:
                        finish_chunk(h, c0, z0)
                        for f in hooks.get(c0, []):
                            f()
                finish_chunk(h, c1, z1)
                for f in hooks.get(c1, []):
                    f()

            def emit_outproj(h, jjs):
                zT, rc, wot = st[h]["zT"], st[h]["rc"], st[h]["wo"]
                for jj in jjs:
                    ob = ob_pool.tile([128, 2, DM], FP16, name=f"ob{h}{jj}",
                                      tag="ob")
                    j0, j1 = 2 * jj, 2 * jj + 1
                    for mo, mw in ((0, 512), (512, 256)):
                        apsA = ps_aux.tile([128, 512], F32,
                                           name=f"o{h}{j0}{mo}", tag="a")
                        apsB = ps_aux.tile([128, 512], F32,
                                           name=f"o{h}{j1}{mo}", tag="a")
                        nc.tensor.matmul(apsA[:, 0:mw],
                                         zT[0:DH, bass.ts(j0, 128)],
                                         wot[0:DH, mo:mo + mw],
                                         start=True, stop=True)
                        nc.tensor.matmul(apsB[:, 0:mw],
                                         zT[DH:128, bass.ts(j1, 128)],
                                         wot[DH:128, mo:mo + mw],
                                         start=True, stop=True)
                        nc.vector.tensor_scalar_mul(
                            ob[:, 0, mo:mo + mw], apsA[:, 0:mw],
                            rc[:, j0:j0 + 1])
                        nc.vector.tensor_scalar_mul(
                            ob[:, 1, mo:mo + mw], apsB[:, 0:mw],
                            rc[:, j1:j1 + 1])
                    ring = nc.gpsimd if jj % 2 == 0 else nc.sync
                    ring.dma_start(
                        out[h, bass.ts(jj, 256), :]
                           .rearrange("(a p) m -> p a m", p=128),
                        ob[:])

            emit_loads(0)
            emit_proj_qk(0)
            emit_proj_v(0)
            emit_vp(0)
            for h in range(HPC):
                nxt, prv = h + 1, h - 1
                if nxt < HPC:
                    emit_loads(nxt)
                acts = {0: [], 1: [], 2: [], 3: []}
                if prv >= 0:
                    acts[0].append(lambda p=prv: emit_outproj(p, (4, 5)))
                    acts[1].append(lambda p=prv: emit_outproj(p, (6, 7)))
                if nxt < HPC:
                    acts[1].append(lambda n=nxt: emit_proj_qk(n))
                    acts[2].append(lambda n=nxt: emit_proj_v(n))
                    acts[3].append(lambda n=nxt: emit_vp(n))
                if debug and h == 0:
                    nc.gpsimd.dma_start(dqT[:], st[0]["qT"][:])
                    nc.gpsimd.dma_start(dkT[:], st[0]["kT"][:])
                    nc.gpsimd.dma_start(dvT[:], st[0]["vT"][:])
                    for i in range(NT):
                        nc.gpsimd.dma_start(dvp[:, i, :], st[0]["vp"][i][:, 0:DH])
                emit_pass(h, (0, 1), {c: acts[c] for c in (0, 1)})
                emit_outproj(h, (0, 1, 2, 3))
                emit_pass(h, (2, 3), {c: acts[c] for c in (2, 3)})
                if debug and h == 0:
                    nc.gpsimd.dma_start(dzT[:], st[0]["zT"][:])
                    nc.gpsimd.dma_start(drc[:], st[0]["rc"][:])
            emit_outproj(HPC - 1, (4, 5, 6, 7))
    nc.compile()
    return nc


_CACHED = None


def _program(debug=False):
    global _CACHED
    if _CACHED is None:
        _CACHED = build_program(debug)
    return _CACHED


def _make_in_maps(inputs):
    xq_f = np.asarray(inputs["normalized_resid_pre_q"], dtype=np.float32)
    xk_f = np.asarray(inputs["normalized_resid_pre_k"], dtype=np.float32)
    xv_f = np.asarray(inputs["normalized_resid_pre_v"], dtype=np.float32)
    WQ = np.asarray(inputs["W_Q"], dtype=np.float32) * WSC
    WK = np.asarray(inputs["W_K"], dtype=np.float32) * WSC
    WV = np.asarray(inputs["W_V"], dtype=np.float32)
    WO = np.asarray(inputs["W_O"], dtype=np.float32)
    bQ = np.asarray(inputs["b_Q"], dtype=np.float32) * WSC
    bK = np.asarray(inputs["b_K"], dtype=np.float32) * WSC
    bV = np.asarray(inputs["b_V"], dtype=np.float32)
    bO = np.asarray(inputs["b_O"], dtype=np.float32)

    def interleave_x(x):  # [DM, S] -> [2, 128, 3, 2, 1024] (s-halves split)
        y = x.reshape(3, 2, 128, 2, 1024)
        return np.ascontiguousarray(y.transpose(3, 2, 0, 1, 4))

    def interleave_w(w):  # [DM, DH] -> [128, 3, 2, DH]
        return np.ascontiguousarray(
            w.reshape(3, 2, 128, DH).transpose(2, 0, 1, 3))

    identh = np.eye(128, dtype=np.float16)
    masku = ((np.arange(128)[:, None] > np.arange(128)[None, :])
             .astype(np.float16) * np.float16(NEG))

    bq2 = np.zeros((H, 128, 1), np.float32)
    bq2[:, 0:DH, 0] = bQ
    bq2[:, DH:128, 0] = bQ
    bk2 = np.zeros((H, 128, 1), np.float32)
    bk2[:, 0:DH, 0] = bK
    bk2[:, DH:128, 0] = bK
    bv2 = np.zeros((H, 128, 1), np.float32)
    bv2[:, 0:DH, 0] = bV
    bv2[:, DH:128, 0] = bV

    in_maps = []
    for c in range(N_CORES):
        b = c % 2
        hg = c // 2
        hs = slice(HPC * hg, HPC * hg + HPC)
        m = {
            "xq": np.stack([interleave_x(
                xq_f[b, :, HPC * hg + j, :].T) for j in range(HPC)]
                ).astype(NP_X),
            "xk": np.stack([interleave_x(
                xk_f[b, :, HPC * hg + j, :].T) for j in range(HPC)]
                ).astype(NP_X),
            "xv": np.ascontiguousarray(
                xv_f[b, :, hs, :].transpose(1, 2, 0)
                .reshape(HPC, MT, 128, 2, 1024).transpose(0, 3, 2, 1, 4)
                ).astype(np.float16),
            "wp8": np.concatenate(sum((
                [interleave_w(WQ[HPC * hg + j]).reshape(128, 384),
                 interleave_w(WK[HPC * hg + j]).reshape(128, 384)]
                for j in range(HPC)), []), axis=1).astype(NP_X),
            "wp16": np.concatenate(sum((
                [WV[HPC * hg + j].reshape(MT, 128, DH)
                 .transpose(1, 0, 2).reshape(128, 384),
                 np.concatenate([WO[HPC * hg + j], WO[HPC * hg + j]],
                                axis=0)]
                for j in range(HPC)), []), axis=1).astype(np.float16),
            "bp": np.concatenate(sum((
                [bq2[HPC * hg + j], bk2[HPC * hg + j], bv2[HPC * hg + j]]
                for j in range(HPC)), []), axis=1).astype(np.float32),
            "identh": identh,
            "masku": masku,
        }
        in_maps.append(m)
    return in_maps


def run(inputs, trace=False, debug=False, **kw):
    nc = _program(debug)
    in_maps = _make_in_maps(inputs)
    res = run_bass_kernel_spmd(nc, in_maps, core_ids=list(range(N_CORES)),
                               trace=trace, **kw)
    full = np.zeros((B, S, H, DM), np.float32)
    for c in range(N_CORES):
        b = c % 2
        hg = c // 2
        o = res.results[c]["out"]
        for j in range(HPC):
            full[b, :, HPC * hg + j, :] = o[j]
    bO = np.asarray(inputs["b_O"], dtype=np.float32)
    if np.any(bO):
        full += bO / H
    return full, res


def kernel(**inputs):
    full, _ = run(inputs)
    return full


# revision 25
# speedup vs baseline: 1.1620x; 1.0963x over previous
"""Trainium2 Bass kernel for per-head causal attention (nn_Attention_52896817217709).

Sharding: 8 cores = 4 head-groups (3 heads each) x 2 batches.
Per core, per head h (S=2048, D_MODEL=768, D_HEAD=64):
  q&k projected together per 512-chunk, packed on the two PE column halves
  (tile_position (0,0)/(0,64)) -> qkT [128,S] (q rows 0:64, k rows 64:128),
  one full-lane DVE evac per chunk; swap-dup into kqT via SBUF->SBUF DMA so
  the scores matmuls can be 2-way row-packed (K=64 halves, concurrent).
  v self-paired on chunk pairs -> checkerboarded vT; vp (PV lhsT [k,d]) via
  XBAR DMA-transpose into offset-0 pool slots (split sync/scalar rings).
  Causal diag-tile masking is an additive PE matmul (identity^T @ maskU).
  Attention runs in two passes over chunk pairs (q 0:1024 then 1024:2048),
  [128,1024] two-bank PSUM score tiles -> one exp per k-tile (scalar engine
  does only exp).  PV z' [64,512] per chunk plus a concurrent col-packed
  M=1 ones-matmul accumulating softmax sums into z row 64.
  out = (z'^T_j @ [W_O; b_O/H]) * rc_j with rc = 1/sums; evac on DVE; fp16 out.
  xq/xk and W_Q/W_K optionally fp8e4m3 (W scaled x16, absorbed in exp scale).
"""
import sys
import os
import numpy as np

for _p in ("/opt/trn_rl_repo", "/root/.axon_site/_ro/trn_rl_repo"):
    if os.path.isdir(_p) and _p not in sys.path:
        sys.path.insert(0, _p)

import ml_dtypes
import concourse.bass as bass
import concourse.tile as tile
from concourse import bacc, mybir
from concourse.bass_utils import run_bass_kernel_spmd

F32 = mybir.dt.float32
FP16 = mybir.dt.float16
FP8 = mybir.dt.float8e4
AF = mybir.ActivationFunctionType

B, S, H, DM, DH = 2, 2048, 12, 768, 64
HPC = 3            # heads per core
NT = S // 128      # 16 k-tiles
MT = DM // 128     # 6 m-tiles
N_CORES = 8
NEG = -60000.0     # additive causal-mask constant (fp16-safe)

USE_FP8 = True     # xq/xk + W_Q/W_K in fp8e4m3 (x16 weight scale)
WSC = 16.0 if USE_FP8 else 1.0
SCALE = 0.125 / (WSC * WSC)   # exp scale absorbs 1/sqrt(DH) and fp8 scaling
XQK_DT = FP8 if USE_FP8 else FP16
NP_X = ml_dtypes.float8_e4m3fn if USE_FP8 else np.float16


def build_program(debug=False):
    nc = bacc.Bacc("TRN2", target_bir_lowering=False, debug=False)

    xq = nc.dram_tensor("xq", [HPC, 2, 128, 3, 2, 1024], XQK_DT,
                        kind="ExternalInput")
    xk = nc.dram_tensor("xk", [HPC, 2, 128, 3, 2, 1024], XQK_DT,
                        kind="ExternalInput")
    xv = nc.dram_tensor("xv", [HPC, 2, 128, MT, 1024], FP16,
                        kind="ExternalInput")
    wp8 = nc.dram_tensor("wp8", [128, HPC * 768], XQK_DT,
                         kind="ExternalInput")
    wp16 = nc.dram_tensor("wp16", [128, HPC * 1152], FP16,
                          kind="ExternalInput")
    bp = nc.dram_tensor("bp", [128, HPC * 3], F32, kind="ExternalInput")
    identh = nc.dram_tensor("identh", [128, 128], FP16, kind="ExternalInput")
    masku = nc.dram_tensor("masku", [128, 128], FP16, kind="ExternalInput")
    out = nc.dram_tensor("out", [HPC, S, DM], FP16, kind="ExternalOutput")
    qscr = nc.dram_tensor("qscr", [HPC, DH, S], FP16, kind="Internal")
    kscr = nc.dram_tensor("kscr", [HPC, DH, S], FP16, kind="Internal")
    zscr = nc.dram_tensor("zscr", [HPC, DH, S], FP16, kind="Internal")
    if debug:
        dqT = nc.dram_tensor("dqT", [128, S], FP16, kind="ExternalOutput")
        dkT = nc.dram_tensor("dkT", [128, S], FP16, kind="ExternalOutput")
        dvT = nc.dram_tensor("dvT", [128, S], FP16, kind="ExternalOutput")
        dvp = nc.dram_tensor("dvp", [128, NT, DH], FP16, kind="ExternalOutput")
        dzT = nc.dram_tensor("dzT", [128, S], FP16, kind="ExternalOutput")
        drc = nc.dram_tensor("drc", [128, NT], F32, kind="ExternalOutput")

    with tile.TileContext(nc) as tc:
        with (
            tc.tile_pool(name="wpool", bufs=1) as wpool,
            tc.tile_pool(name="xp", bufs=4) as x_pool,
            tc.tile_pool(name="wt", bufs=2) as wt_pool,
            tc.tile_pool(name="qk", bufs=2) as qk_pool,
            tc.tile_pool(name="vp", bufs=24) as vp_pool,
            tc.tile_pool(name="pp", bufs=6) as p_pool,
            tc.tile_pool(name="zt", bufs=2) as zt_pool,
            tc.tile_pool(name="rc", bufs=2) as rc_pool,
            tc.tile_pool(name="ob", bufs=3) as ob_pool,
            tc.tile_pool(name="psa", bufs=2, space="PSUM") as ps_aux,
            tc.tile_pool(name="pss", bufs=2, space="PSUM") as ps_s,
            tc.tile_pool(name="psz", bufs=2, space="PSUM") as ps_z,
        ):
            id_sb = wpool.tile([128, 128], FP16, name="id_sb")
            nc.gpsimd.dma_start(id_sb[:], identh[:])
            mask_sb = wpool.tile([128, 128], FP16, name="mask_sb")
            nc.gpsimd.dma_start(mask_sb[:], masku[:])
            ones_sb = wpool.tile([128, 1], FP16, name="ones_sb")
            nc.gpsimd.memset(ones_sb[:], 1.0)
            w8_sb = wpool.tile([128, HPC * 768], XQK_DT, name="w8_sb")
            nc.sync.dma_start(w8_sb[:], wp8[:])
            w16_sb = wpool.tile([128, HPC * 1152], FP16, name="w16_sb")
            nc.sync.dma_start(w16_sb[:], wp16[:])
            bp_sb = wpool.tile([128, HPC * 3], F32, name="bp_sb")
            nc.sync.dma_start(bp_sb[:], bp[:])

            st = [dict() for _ in range(HPC)]
            for _h in range(HPC):
                st[_h]["wq"] = w8_sb[:, 768 * _h:768 * _h + 384].rearrange(
                    "p (b k d) -> p b k d", b=3, k=2)
                st[_h]["wk"] = w8_sb[:, 768 * _h + 384:768 * _h + 768
                                     ].rearrange("p (b k d) -> p b k d",
                                                 b=3, k=2)
                st[_h]["wv"] = w16_sb[:, 1152 * _h:1152 * _h + 384
                                      ].rearrange("p (a d) -> p a d", a=MT)
                st[_h]["wo"] = w16_sb[:, 1152 * _h + 384:1152 * _h + 1152]
                st[_h]["bq"] = bp_sb[:, 3 * _h:3 * _h + 1]
                st[_h]["bk"] = bp_sb[:, 3 * _h + 1:3 * _h + 2]
                st[_h]["bv"] = bp_sb[:, 3 * _h + 2:3 * _h + 3]

            def emit_loads(h):
                """xq/xk halves on sync; xv on gpsimd."""
                for t, xd in (("q", xq), ("k", xk)):
                    halves = []
                    for a in range(2):
                        xt = x_pool.tile([128, 3, 2, 1024], XQK_DT,
                                         name=f"x{t}{h}{a}", tag=f"x{t}")
                        nc.sync.dma_start(xt[:], xd[h, a])
                        halves.append(xt)
                    st[h][f"x{t}"] = halves
                halves = []
                for a in range(2):
                    xt = x_pool.tile([128, MT, 1024], FP16,
                                     name=f"xv{h}{a}", tag="xv")
                    nc.gpsimd.dma_start(xt[:], xv[h, a])
                    halves.append(xt)
                st[h]["xv"] = halves

            def emit_proj_qk(h):
                """q,k DoubleRow projections -> qT/kT, dup'd to both halves."""
                qT = qk_pool.tile([128, S], FP16, name=f"qT{h}", tag="qT")
                kT = qk_pool.tile([128, S], FP16, name=f"kT{h}", tag="kT")
                st[h]["qT"], st[h]["kT"] = qT, kT
                DR = mybir.MatmulPerfMode.DoubleRow
                for c in range(4):
                    off = (c % 2) * 512
                    for t, dst in (("q", qT), ("k", kT)):
                        xt = st[h][f"x{t}"][c // 2]
                        wt = st[h][f"w{t}"]
                        b = st[h][f"b{t}"]
                        acc = ps_aux.tile([128, 512], F32,
                                          name=f"a{t}{h}{c}", tag="a")
                        for bb in range(3):
                            nc.tensor.matmul(
                                acc[0:DH, :], wt[:, bb, :, :],
                                xt[:, bb, :, off:off + 512],
                                start=(bb == 0), stop=(bb == 2),
                                perf_mode=DR)
                        nc.vector.tensor_scalar_add(
                            dst[0:DH, bass.ts(c, 512)], acc[0:DH, :],
                            b[0:DH])
                        ring = nc.sync if t == "q" else nc.gpsimd
                        ring.dma_start(dst[DH:128, bass.ts(c, 512)],
                                       dst[0:DH, bass.ts(c, 512)])

            def emit_proj_v(h):
                """v self-paired on chunk pairs -> checkerboarded vT."""
                vT = qk_pool.tile([128, S], FP16, name=f"vT{h}", tag="vT")
                st[h]["vT"] = vT
                w, b = st[h]["wv"], st[h]["bv"]
                for pr in range(2):
                    xt = st[h]["xv"][pr]
                    acc = ps_aux.tile([128, 512], F32, name=f"av{h}{pr}",
                                      tag="a")
                    for mt in range(MT):
                        nc.tensor.matmul(
                            acc[0:DH, :], w[:, mt, :], xt[:, mt, 0:512],
                            start=(mt == 0), stop=(mt == MT - 1),
                            tile_position=(0, 0))
                        nc.tensor.matmul(
                            acc[DH:128, :], w[:, mt, :], xt[:, mt, 512:1024],
                            start=(mt == 0), stop=(mt == MT - 1),
                            tile_position=(0, DH))
                    c0, c1 = 2 * pr, 2 * pr + 1
                    nc.vector.tensor_scalar_add(
                        vT[0:DH, bass.ts(c0, 512)], acc[0:DH, :], b[0:DH])
                    nc.vector.tensor_scalar_add(
                        vT[DH:128, bass.ts(c1, 512)], acc[DH:128, :],
                        b[DH:128])

            def emit_vp(h):
                """PV lhsT [k, d|1] per k-tile via PE transpose + DVE copy."""
                vT = st[h]["vT"]
                vps = []
                for i in range(NT):
                    r0 = 0 if (i // 4) % 2 == 0 else DH
                    vt = vp_pool.tile([128, DH + 4], FP16, name=f"vp{h}_{i}",
                                      tag="vp")
                    v_ps = ps_aux.tile([128, DH], FP16, name=f"vps{h}{i}",
                                       tag="a", padded_shape=[128, 1024])
                    nc.tensor.transpose(v_ps[:], vT[r0:r0 + DH, bass.ts(i, 128)],
                                        id_sb[r0:r0 + DH, r0:r0 + DH])
                    nc.vector.tensor_copy(vt[:, 0:DH], v_ps[:])
                    nc.gpsimd.memset(vt[:, DH:DH + 1], 1.0)
                    vps.append(vt)
                st[h]["vp"] = vps

            def stage_pair(h, i0, qhi):
                """Scores + exp for k-tiles i0, i0+1 (row-packed halves)."""
                qT, kT = st[h]["qT"], st[h]["kT"]
                res = []
                for i, pos in ((i0, 0), (i0 + 1, DH)):
                    qlo = max(128 * i, qhi - 1024)
                    w = qhi - qlo
                    sp = ps_s.tile([128, 1024], F32, name=f"s{h}{i}{qhi}",
                                   tag="s")
                    diag = qlo == 128 * i
                    kt = kT[pos:pos + DH, bass.ts(i, 128)]
                    qt = qT
                    for o in range(0, w, 512):
                        ww = min(512, w - o)
                        nc.tensor.matmul(sp[:, o:o + ww], kt,
                                         qt[pos:pos + DH,
                                            qlo + o:qlo + o + ww],
                                         start=True,
                                         stop=not (diag and o == 0))
                        if diag and o == 0:
                            nc.tensor.matmul(sp[:, 0:128], id_sb[:],
                                             mask_sb[:], start=False,
                                             stop=True)
                    P = p_pool.tile([128, 1024], FP16, name=f"P{h}{i}{qhi}",
                                    tag="P")
                    nc.scalar.activation(P[:, 0:w], sp[:, 0:w], AF.Exp,
                                         scale=SCALE)
                    res.append((P, qlo))
                return res

            def finish_chunk(h, c, zps):
                zT, rc, srow = st[h]["zT"], st[h]["rc"], st[h]["srow"]
                nc.vector.tensor_copy(zT[0:DH, bass.ts(c, 512)], zps[0:DH, :])
                nc.vector.tensor_copy(srow[DH:DH + 1, bass.ts(c, 512)],
                                      zps[DH:DH + 1, :])
                rcp = ps_aux.tile([128, 8], FP16, name=f"rcp{h}{c}", tag="a",
                                  padded_shape=[128, 1024])
                for j in range(4):
                    nc.tensor.transpose(
                        rcp[:, 2 * j:2 * j + 1],
                        srow[DH:DH + 1,
                             512 * c + 128 * j:512 * c + 128 * j + 128],
                        id_sb[DH:DH + 1, DH:DH + 1])
                nc.vector.reciprocal(rc[:, 4 * c:4 * c + 4], rcp[:, 0:8:2])
                nc.gpsimd.dma_start(zT[DH:128, bass.ts(c, 512)],
                                    zT[0:DH, bass.ts(c, 512)])

            def emit_pass(h, cpair, hooks):
                """Attention pass over chunks cpair=(c0,c1); i-major PVs."""
                c0, c1 = cpair
                qhi = 512 * c1 + 512
                nk = 4 * c1 + 4
                vp = st[h]["vp"]
                if c0 == 0:
                    zT = zt_pool.tile([128, S], FP16, name=f"zT{h}", tag="zT")
                    rc = rc_pool.tile([128, NT], F32, name=f"rc{h}", tag="rc")
                    srow = rc_pool.tile([DH + 1, S], FP16, name=f"srow{h}",
                                        tag="srow")
                    st[h]["srow"] = srow
                    st[h]["zT"] = zT
                    st[h]["rc"] = rc
                z0 = ps_z.tile([DH + 1, 512], F32, name=f"z{h}{c0}", tag="z")
                z1 = ps_z.tile([DH + 1, 512], F32, name=f"z{h}{c1}", tag="z")
                staged = {}
                for i0 in (0, 2):
                    for P, j in zip(stage_pair(h, i0, qhi), (i0, i0 + 1)):
                        staged[j] = P
                for i in range(nk):
                    if i % 2 == 0 and i + 4 < nk:
                        for P, j in zip(stage_pair(h, i + 4, qhi),
                                        (i + 4, i + 5)):
                            staged[j] = P
                    P, qlo = staged[i]
                    for c, z in ((c0, z0), (c1, z1)):
                        if i >= 4 * c + 4:
                            continue
                        ql = max(512 * c, 128 * i)
                        w = 512 * c + 512 - ql
                        zc = ql - 512 * c
                        Pc = P[:, ql - qlo:ql - qlo + w]
                        nc.tensor.matmul(
                            z[:, zc:zc + w], vp[i][:, 0:DH + 1], Pc,
                            start=(i == 0), stop=(i == 4 * c + 3))
                    del staged[i]
                    if i == 4 * c0 + 3:
                        finish_chunk(h, c0, z0)
                        for f in hooks.get(c0, []):
                            f()
                finish_chunk(h, c1, z1)
                for f in hooks.get(c1, []):
                    f()

            def emit_outproj(h, jjs):
                zT, rc, wot = st[h]["zT"], st[h]["rc"], st[h]["wo"]
                for jj in jjs:
                    ob = ob_pool.tile([128, 2, DM], FP16, name=f"ob{h}{jj}",
                                      tag="ob")
                    j0, j1 = 2 * jj, 2 * jj + 1
                    for mo, mw in ((0, 512), (512, 256)):
                        apsA = ps_aux.tile([128, 512], F32,
                                           name=f"o{h}{j0}{mo}", tag="a")
                        apsB = ps_aux.tile([128, 512], F32,
                                           name=f"o{h}{j1}{mo}", tag="a")
                        nc.tensor.matmul(apsA[:, 0:mw],
                                         zT[0:DH, bass.ts(j0, 128)],
                                         wot[0:DH, mo:mo + mw],
                                         start=True, stop=True)
                        nc.tensor.matmul(apsB[:, 0:mw],
                                         zT[DH:128, bass.ts(j1, 128)],
                                         wot[DH:128, mo:mo + mw],
                                         start=True, stop=True)
                        nc.vector.tensor_scalar_mul(
                            ob[:, 0, mo:mo + mw], apsA[:, 0:mw],
                            rc[:, j0:j0 + 1])
                        nc.vector.tensor_scalar_mul(
                            ob[:, 1, mo:mo + mw], apsB[:, 0:mw],
                            rc[:, j1:j1 + 1])
                    nc.scalar.dma_start(
                        out[h, bass.ts(jj, 256), :]
                           .rearrange("(a p) m -> p a m", p=128),
                        ob[:])

            emit_loads(0)
            emit_proj_qk(0)
            emit_proj_v(0)
            emit_vp(0)
            for h in range(HPC):
                nxt, prv = h + 1, h - 1
                if nxt < HPC:
                    emit_loads(nxt)
                acts = {0: [], 1: [], 2: [], 3: []}
                if prv >= 0:
                    acts[0].append(lambda p=prv: emit_outproj(p, (4, 5)))
                    acts[1].append(lambda p=prv: emit_outproj(p, (6, 7)))
                if nxt < HPC:
                    acts[1].append(lambda n=nxt: emit_proj_qk(n))
                    acts[2].append(lambda n=nxt: emit_proj_v(n))
                    acts[3].append(lambda n=nxt: emit_vp(n))
                if debug and h == 0:
                    nc.gpsimd.dma_start(dqT[:], st[0]["qT"][:])
                    nc.gpsimd.dma_start(dkT[:], st[0]["kT"][:])
                    nc.gpsimd.dma_start(dvT[:], st[0]["vT"][:])
                    for i in range(NT):
                        nc.gpsimd.dma_start(dvp[:, i, :], st[0]["vp"][i][:, 0:DH])
                emit_pass(h, (0, 1), {c: acts[c] for c in (0, 1)})
                emit_outproj(h, (0, 1, 2, 3))
                emit_pass(h, (2, 3), {c: acts[c] for c in (2, 3)})
                if debug and h == 0:
                    nc.gpsimd.dma_start(dzT[:], st[0]["zT"][:])
                    nc.gpsimd.dma_start(drc[:], st[0]["rc"][:])
            emit_outproj(HPC - 1, (4, 5, 6, 7))
    nc.compile()
    return nc


_CACHED = None


def _program(debug=False):
    global _CACHED
    if _CACHED is None:
        _CACHED = build_program(debug)
    return _CACHED


def _make_in_maps(inputs):
    xq_f = np.asarray(inputs["normalized_resid_pre_q"], dtype=np.float32)
    xk_f = np.asarray(inputs["normalized_resid_pre_k"], dtype=np.float32)
    xv_f = np.asarray(inputs["normalized_resid_pre_v"], dtype=np.float32)
    WQ = np.asarray(inputs["W_Q"], dtype=np.float32) * WSC
    WK = np.asarray(inputs["W_K"], dtype=np.float32) * WSC
    WV = np.asarray(inputs["W_V"], dtype=np.float32)
    WO = np.asarray(inputs["W_O"], dtype=np.float32)
    bQ = np.asarray(inputs["b_Q"], dtype=np.float32) * WSC
    bK = np.asarray(inputs["b_K"], dtype=np.float32) * WSC
    bV = np.asarray(inputs["b_V"], dtype=np.float32)
    bO = np.asarray(inputs["b_O"], dtype=np.float32)

    def interleave_x(x):  # [DM, S] -> [2, 128, 3, 2, 1024] (s-halves split)
        y = x.reshape(3, 2, 128, 2, 1024)
        return np.ascontiguousarray(y.transpose(3, 2, 0, 1, 4))

    def interleave_w(w):  # [DM, DH] -> [128, 3, 2, DH]
        return np.ascontiguousarray(
            w.reshape(3, 2, 128, DH).transpose(2, 0, 1, 3))

    identh = np.eye(128, dtype=np.float16)
    masku = ((np.arange(128)[:, None] > np.arange(128)[None, :])
             .astype(np.float16) * np.float16(NEG))

    bq2 = np.zeros((H, 128, 1), np.float32)
    bq2[:, 0:DH, 0] = bQ
    bq2[:, DH:128, 0] = bQ
    bk2 = np.zeros((H, 128, 1), np.float32)
    bk2[:, 0:DH, 0] = bK
    bk2[:, DH:128, 0] = bK
    bv2 = np.zeros((H, 128, 1), np.float32)
    bv2[:, 0:DH, 0] = bV
    bv2[:, DH:128, 0] = bV

    in_maps = []
    for c in range(N_CORES):
        b = c % 2
        hg = c // 2
        hs = slice(HPC * hg, HPC * hg + HPC)
        m = {
            "xq": np.stack([interleave_x(
                xq_f[b, :, HPC * hg + j, :].T) for j in range(HPC)]
                ).astype(NP_X),
            "xk": np.stack([interleave_x(
                xk_f[b, :, HPC * hg + j, :].T) for j in range(HPC)]
                ).astype(NP_X),
            "xv": np.ascontiguousarray(
                xv_f[b, :, hs, :].transpose(1, 2, 0)
                .reshape(HPC, MT, 128, 2, 1024).transpose(0, 3, 2, 1, 4)
                ).astype(np.float16),
            "wp8": np.concatenate(sum((
                [interleave_w(WQ[HPC * hg + j]).reshape(128, 384),
                 interleave_w(WK[HPC * hg + j]).reshape(128, 384)]
                for j in range(HPC)), []), axis=1).astype(NP_X),
            "wp16": np.concatenate(sum((
                [WV[HPC * hg + j].reshape(MT, 128, DH)
                 .transpose(1, 0, 2).reshape(128, 384),
                 np.concatenate([WO[HPC * hg + j], WO[HPC * hg + j]],
                                axis=0)]
                for j in range(HPC)), []), axis=1).astype(np.float16),
            "bp": np.concatenate(sum((
                [bq2[HPC * hg + j], bk2[HPC * hg + j], bv2[HPC * hg + j]]
                for j in range(HPC)), []), axis=1).astype(np.float32),
            "identh": identh,
            "masku": masku,
        }
        in_maps.append(m)
    return in_maps


def run(inputs, trace=False, debug=False, **kw):
    nc = _program(debug)
    in_maps = _make_in_maps(inputs)
    res = run_bass_kernel_spmd(nc, in_maps, core_ids=list(range(N_CORES)),
                               trace=trace, **kw)
    full = np.zeros((B, S, H, DM), np.float32)
    for c in range(N_CORES):
        b = c % 2
        hg = c // 2
        o = res.results[c]["out"]
        for j in range(HPC):
            full[b, :, HPC * hg + j, :] = o[j]
    bO = np.asarray(inputs["b_O"], dtype=np.float32)
    if np.any(bO):
        full += bO / H
    return full, res


def kernel(**inputs):
    full, _ = run(inputs)
    return full


# revision 26
# speedup vs baseline: 1.2641x; 1.0879x over previous
"""Trainium2 Bass kernel for per-head causal attention (nn_Attention_52896817217709).

Sharding: 8 cores = 4 head-groups (3 heads each) x 2 batches.
Per core, per head h (S=2048, D_MODEL=768, D_HEAD=64):
  q&k projected together per 512-chunk, packed on the two PE column halves
  (tile_position (0,0)/(0,64)) -> qkT [128,S] (q rows 0:64, k rows 64:128),
  one full-lane DVE evac per chunk; swap-dup into kqT via SBUF->SBUF DMA so
  the scores matmuls can be 2-way row-packed (K=64 halves, concurrent).
  v self-paired on chunk pairs -> checkerboarded vT; vp (PV lhsT [k,d]) via
  XBAR DMA-transpose into offset-0 pool slots (split sync/scalar rings).
  Causal diag-tile masking is an additive PE matmul (identity^T @ maskU).
  Attention runs in two passes over chunk pairs (q 0:1024 then 1024:2048),
  [128,1024] two-bank PSUM score tiles -> one exp per k-tile (scalar engine
  does only exp).  PV z' [64,512] per chunk plus a concurrent col-packed
  M=1 ones-matmul accumulating softmax sums into z row 64.
  out = (z'^T_j @ [W_O; b_O/H]) * rc_j with rc = 1/sums; evac on DVE; fp16 out.
  xq/xk and W_Q/W_K optionally fp8e4m3 (W scaled x16, absorbed in exp scale).
"""
import sys
import os
import numpy as np

for _p in ("/opt/trn_rl_repo", "/root/.axon_site/_ro/trn_rl_repo"):
    if os.path.isdir(_p) and _p not in sys.path:
        sys.path.insert(0, _p)

import ml_dtypes
import concourse.bass as bass
import concourse.tile as tile
from concourse import bacc, mybir
from concourse.bass_utils import run_bass_kernel_spmd

F32 = mybir.dt.float32
FP16 = mybir.dt.float16
FP8 = mybir.dt.float8e4
AF = mybir.ActivationFunctionType

B, S, H, DM, DH = 2, 2048, 12, 768, 64
HPC = 3            # heads per core
NT = S // 128      # 16 k-tiles
MT = DM // 128     # 6 m-tiles
N_CORES = 8
NEG = -60000.0     # additive causal-mask constant (fp16-safe)

USE_FP8 = True     # xq/xk + W_Q/W_K in fp8e4m3 (x16 weight scale)
WSC = 16.0 if USE_FP8 else 1.0
SCALE = 0.125 / (WSC * WSC)   # exp scale absorbs 1/sqrt(DH) and fp8 scaling
XQK_DT = FP8 if USE_FP8 else FP16
NP_X = ml_dtypes.float8_e4m3fn if USE_FP8 else np.float16


def build_program(debug=False):
    nc = bacc.Bacc("TRN2", target_bir_lowering=False, debug=False)

    xq = nc.dram_tensor("xq", [HPC, 2, 128, 3, 2, 1024], XQK_DT,
                        kind="ExternalInput")
    xk = nc.dram_tensor("xk", [HPC, 2, 128, 3, 2, 1024], XQK_DT,
                        kind="ExternalInput")
    xv = nc.dram_tensor("xv", [HPC, 2, 128, MT, 1024], FP16,
                        kind="ExternalInput")
    wp8 = nc.dram_tensor("wp8", [128, HPC * 768], XQK_DT,
                         kind="ExternalInput")
    wp16 = nc.dram_tensor("wp16", [128, HPC * 1152], FP16,
                          kind="ExternalInput")
    bp = nc.dram_tensor("bp", [128, HPC * 3], F32, kind="ExternalInput")
    identh = nc.dram_tensor("identh", [128, 128], FP16, kind="ExternalInput")
    masku = nc.dram_tensor("masku", [128, 128], FP16, kind="ExternalInput")
    out = nc.dram_tensor("out", [HPC, S, DM], FP16, kind="ExternalOutput")
    qscr = nc.dram_tensor("qscr", [HPC, DH, S], FP16, kind="Internal")
    kscr = nc.dram_tensor("kscr", [HPC, DH, S], FP16, kind="Internal")
    zscr = nc.dram_tensor("zscr", [HPC, DH, S], FP16, kind="Internal")
    if debug:
        dqT = nc.dram_tensor("dqT", [128, S], FP16, kind="ExternalOutput")
        dkT = nc.dram_tensor("dkT", [128, S], FP16, kind="ExternalOutput")
        dvT = nc.dram_tensor("dvT", [128, S], FP16, kind="ExternalOutput")
        dvp = nc.dram_tensor("dvp", [128, NT, DH], FP16, kind="ExternalOutput")
        dzT = nc.dram_tensor("dzT", [128, S], FP16, kind="ExternalOutput")
        drc = nc.dram_tensor("drc", [128, NT], F32, kind="ExternalOutput")

    with tile.TileContext(nc) as tc:
        with (
            tc.tile_pool(name="wpool", bufs=1) as wpool,
            tc.tile_pool(name="xp", bufs=4) as x_pool,
            tc.tile_pool(name="wt", bufs=2) as wt_pool,
            tc.tile_pool(name="qk", bufs=2) as qk_pool,
            tc.tile_pool(name="vp", bufs=24) as vp_pool,
            tc.tile_pool(name="pp", bufs=8) as p_pool,
            tc.tile_pool(name="zt", bufs=2) as zt_pool,
            tc.tile_pool(name="rc", bufs=2) as rc_pool,
            tc.tile_pool(name="ob", bufs=3) as ob_pool,
            tc.tile_pool(name="psa", bufs=2, space="PSUM") as ps_aux,
            tc.tile_pool(name="pss", bufs=2, space="PSUM") as ps_s,
            tc.tile_pool(name="psz", bufs=2, space="PSUM") as ps_z,
        ):
            id_sb = wpool.tile([128, 128], FP16, name="id_sb")
            nc.gpsimd.dma_start(id_sb[:], identh[:])
            mask_sb = wpool.tile([128, 128], FP16, name="mask_sb")
            nc.gpsimd.dma_start(mask_sb[:], masku[:])
            ones_sb = wpool.tile([128, 1], FP16, name="ones_sb")
            nc.gpsimd.memset(ones_sb[:], 1.0)
            w8_sb = wpool.tile([128, HPC * 768], XQK_DT, name="w8_sb")
            nc.sync.dma_start(w8_sb[:], wp8[:])
            w16_sb = wpool.tile([128, HPC * 1152], FP16, name="w16_sb")
            nc.sync.dma_start(w16_sb[:], wp16[:])
            bp_sb = wpool.tile([128, HPC * 3], F32, name="bp_sb")
            nc.sync.dma_start(bp_sb[:], bp[:])

            st = [dict() for _ in range(HPC)]
            for _h in range(HPC):
                st[_h]["wq"] = w8_sb[:, 768 * _h:768 * _h + 384].rearrange(
                    "p (b k d) -> p b k d", b=3, k=2)
                st[_h]["wk"] = w8_sb[:, 768 * _h + 384:768 * _h + 768
                                     ].rearrange("p (b k d) -> p b k d",
                                                 b=3, k=2)
                st[_h]["wv"] = w16_sb[:, 1152 * _h:1152 * _h + 384
                                      ].rearrange("p (a d) -> p a d", a=MT)
                st[_h]["wo"] = w16_sb[:, 1152 * _h + 384:1152 * _h + 1152]
                st[_h]["bq"] = bp_sb[:, 3 * _h:3 * _h + 1]
                st[_h]["bk"] = bp_sb[:, 3 * _h + 1:3 * _h + 2]
                st[_h]["bv"] = bp_sb[:, 3 * _h + 2:3 * _h + 3]

            def emit_loads(h):
                """xq/xk halves on sync; xv on gpsimd."""
                for t, xd in (("q", xq), ("k", xk)):
                    halves = []
                    for a in range(2):
                        xt = x_pool.tile([128, 3, 2, 1024], XQK_DT,
                                         name=f"x{t}{h}{a}", tag=f"x{t}")
                        nc.sync.dma_start(xt[:], xd[h, a])
                        halves.append(xt)
                    st[h][f"x{t}"] = halves
                halves = []
                for a in range(2):
                    xt = x_pool.tile([128, MT, 1024], FP16,
                                     name=f"xv{h}{a}", tag="xv")
                    nc.gpsimd.dma_start(xt[:], xv[h, a])
                    halves.append(xt)
                st[h]["xv"] = halves

            def emit_proj_qk(h):
                """q,k DoubleRow projections -> qT/kT, dup'd to both halves."""
                qT = qk_pool.tile([128, S], FP16, name=f"qT{h}", tag="qT")
                kT = qk_pool.tile([128, S], FP16, name=f"kT{h}", tag="kT")
                st[h]["qT"], st[h]["kT"] = qT, kT
                DR = mybir.MatmulPerfMode.DoubleRow
                for c in range(4):
                    off = (c % 2) * 512
                    for t, dst in (("q", qT), ("k", kT)):
                        xt = st[h][f"x{t}"][c // 2]
                        wt = st[h][f"w{t}"]
                        b = st[h][f"b{t}"]
                        acc = ps_aux.tile([128, 512], F32,
                                          name=f"a{t}{h}{c}", tag="a")
                        for bb in range(3):
                            nc.tensor.matmul(
                                acc[0:DH, :], wt[:, bb, :, :],
                                xt[:, bb, :, off:off + 512],
                                start=(bb == 0), stop=(bb == 2),
                                perf_mode=DR)
                        nc.vector.tensor_scalar_add(
                            dst[0:DH, bass.ts(c, 512)], acc[0:DH, :],
                            b[0:DH])
                        ring = nc.sync if t == "q" else nc.gpsimd
                        ring.dma_start(dst[DH:128, bass.ts(c, 512)],
                                       dst[0:DH, bass.ts(c, 512)])

            def emit_proj_v(h):
                """v self-paired on chunk pairs -> checkerboarded vT."""
                vT = qk_pool.tile([128, S], FP16, name=f"vT{h}", tag="vT")
                st[h]["vT"] = vT
                w, b = st[h]["wv"], st[h]["bv"]
                for pr in range(2):
                    xt = st[h]["xv"][pr]
                    acc = ps_aux.tile([128, 512], F32, name=f"av{h}{pr}",
                                      tag="a")
                    for mt in range(MT):
                        nc.tensor.matmul(
                            acc[0:DH, :], w[:, mt, :], xt[:, mt, 0:512],
                            start=(mt == 0), stop=(mt == MT - 1),
                            tile_position=(0, 0))
                        nc.tensor.matmul(
                            acc[DH:128, :], w[:, mt, :], xt[:, mt, 512:1024],
                            start=(mt == 0), stop=(mt == MT - 1),
                            tile_position=(0, DH))
                    c0, c1 = 2 * pr, 2 * pr + 1
                    nc.vector.tensor_scalar_add(
                        vT[0:DH, bass.ts(c0, 512)], acc[0:DH, :], b[0:DH])
                    nc.vector.tensor_scalar_add(
                        vT[DH:128, bass.ts(c1, 512)], acc[DH:128, :],
                        b[DH:128])

            def emit_vp(h):
                """PV lhsT [k, d|1] per k-tile via PE transpose + DVE copy."""
                vT = st[h]["vT"]
                vps = []
                for i in range(NT):
                    r0 = 0 if (i // 4) % 2 == 0 else DH
                    vt = vp_pool.tile([128, DH + 4], FP16, name=f"vp{h}_{i}",
                                      tag="vp")
                    v_ps = ps_aux.tile([128, DH], FP16, name=f"vps{h}{i}",
                                       tag="a", padded_shape=[128, 1024])
                    nc.tensor.transpose(v_ps[:], vT[r0:r0 + DH, bass.ts(i, 128)],
                                        id_sb[r0:r0 + DH, r0:r0 + DH])
                    nc.vector.tensor_copy(vt[:, 0:DH], v_ps[:])
                    nc.gpsimd.memset(vt[:, DH:DH + 1], 1.0)
                    vps.append(vt)
                st[h]["vp"] = vps

            def stage_pair(h, i0, qhi):
                """Scores + exp for k-tiles i0, i0+1 (row-packed halves)."""
                qT, kT = st[h]["qT"], st[h]["kT"]
                res = []
                for i, pos in ((i0, 0), (i0 + 1, DH)):
                    qlo = max(128 * i, qhi - 1024)
                    w = qhi - qlo
                    sp = ps_s.tile([128, 1024], F32, name=f"s{h}{i}{qhi}",
                                   tag="s")
                    diag = qlo == 128 * i
                    kt = kT[pos:pos + DH, bass.ts(i, 128)]
                    qt = qT
                    for o in range(0, w, 512):
                        ww = min(512, w - o)
                        nc.tensor.matmul(sp[:, o:o + ww], kt,
                                         qt[pos:pos + DH,
                                            qlo + o:qlo + o + ww],
                                         start=True,
                                         stop=not (diag and o == 0))
                        if diag and o == 0:
                            nc.tensor.matmul(sp[:, 0:128], id_sb[:],
                                             mask_sb[:], start=False,
                                             stop=True)
                    P = p_pool.tile([128, 1024], FP16, name=f"P{h}{i}{qhi}",
                                    tag="P")
                    nc.scalar.activation(P[:, 0:w], sp[:, 0:w], AF.Exp,
                                         scale=SCALE)
                    res.append((P, qlo))
                return res

            def finish_chunk(h, c, zps):
                zT, rc, srow = st[h]["zT"], st[h]["rc"], st[h]["srow"]
                nc.vector.tensor_copy(zT[0:DH, bass.ts(c, 512)], zps[0:DH, :])
                nc.vector.tensor_copy(srow[DH:DH + 1, bass.ts(c, 512)],
                                      zps[DH:DH + 1, :])
                rcp = ps_aux.tile([128, 8], FP16, name=f"rcp{h}{c}", tag="a",
                                  padded_shape=[128, 1024])
                for j in range(4):
                    nc.tensor.transpose(
                        rcp[:, 2 * j:2 * j + 1],
                        srow[DH:DH + 1,
                             512 * c + 128 * j:512 * c + 128 * j + 128],
                        id_sb[DH:DH + 1, DH:DH + 1])
                nc.vector.reciprocal(rc[:, 4 * c:4 * c + 4], rcp[:, 0:8:2])
                nc.gpsimd.dma_start(zT[DH:128, bass.ts(c, 512)],
                                    zT[0:DH, bass.ts(c, 512)])

            def emit_pass(h, cpair, hooks):
                """Attention pass over chunks cpair=(c0,c1); i-major PVs."""
                c0, c1 = cpair
                qhi = 512 * c1 + 512
                nk = 4 * c1 + 4
                vp = st[h]["vp"]
                if c0 == 0:
                    zT = zt_pool.tile([128, S], FP16, name=f"zT{h}", tag="zT")
                    rc = rc_pool.tile([128, NT], F32, name=f"rc{h}", tag="rc")
                    srow = rc_pool.tile([DH + 1, S], FP16, name=f"srow{h}",
                                        tag="srow")
                    st[h]["srow"] = srow
                    st[h]["zT"] = zT
                    st[h]["rc"] = rc
                z0 = ps_z.tile([DH + 1, 512], F32, name=f"z{h}{c0}", tag="z")
                z1 = ps_z.tile([DH + 1, 512], F32, name=f"z{h}{c1}", tag="z")
                staged = {}
                for i0 in (0, 2, 4):
                    if i0 < nk:
                        for P, j in zip(stage_pair(h, i0, qhi),
                                        (i0, i0 + 1)):
                            staged[j] = P
                for i in range(nk):
                    if i % 2 == 0 and i + 6 < nk:
                        for P, j in zip(stage_pair(h, i + 6, qhi),
                                        (i + 6, i + 7)):
                            staged[j] = P
                    P, qlo = staged[i]
                    for c, z in ((c0, z0), (c1, z1)):
                        if i >= 4 * c + 4:
                            continue
                        ql = max(512 * c, 128 * i)
                        w = 512 * c + 512 - ql
                        zc = ql - 512 * c
                        Pc = P[:, ql - qlo:ql - qlo + w]
                        nc.tensor.matmul(
                            z[:, zc:zc + w], vp[i][:, 0:DH + 1], Pc,
                            start=(i == 0), stop=(i == 4 * c + 3))
                    del staged[i]
                    if i == 4 * c0 + 3:
                        finish_chunk(h, c0, z0)
                        for f in hooks.get(c0, []):
                            f()
                finish_chunk(h, c1, z1)
                for f in hooks.get(c1, []):
                    f()

            def emit_outproj(h, jjs):
                zT, rc, wot = st[h]["zT"], st[h]["rc"], st[h]["wo"]
                for jj in jjs:
                    ob = ob_pool.tile([128, 2, DM], FP16, name=f"ob{h}{jj}",
                                      tag="ob")
                    j0, j1 = 2 * jj, 2 * jj + 1
                    for mo, mw in ((0, 512), (512, 256)):
                        apsA = ps_aux.tile([128, 512], F32,
                                           name=f"o{h}{j0}{mo}", tag="a")
                        apsB = ps_aux.tile([128, 512], F32,
                                           name=f"o{h}{j1}{mo}", tag="a")
                        nc.tensor.matmul(apsA[:, 0:mw],
                                         zT[0:DH, bass.ts(j0, 128)],
                                         wot[0:DH, mo:mo + mw],
                                         start=True, stop=True)
                        nc.tensor.matmul(apsB[:, 0:mw],
                                         zT[DH:128, bass.ts(j1, 128)],
                                         wot[DH:128, mo:mo + mw],
                                         start=True, stop=True)
                        nc.vector.tensor_scalar_mul(
                            ob[:, 0, mo:mo + mw], apsA[:, 0:mw],
                            rc[:, j0:j0 + 1])
                        nc.vector.tensor_scalar_mul(
                            ob[:, 1, mo:mo + mw], apsB[:, 0:mw],
                            rc[:, j1:j1 + 1])
                    nc.scalar.dma_start(
                        out[h, bass.ts(jj, 256), :]
                           .rearrange("(a p) m -> p a m", p=128),
                        ob[:])

            emit_loads(0)
            emit_proj_qk(0)
            emit_proj_v(0)
            emit_vp(0)
            for h in range(HPC):
                nxt, prv = h + 1, h - 1
                if nxt < HPC:
                    emit_loads(nxt)
                acts = {0: [], 1: [], 2: [], 3: []}
                if prv >= 0:
                    acts[0].append(lambda p=prv: emit_outproj(p, (4, 5)))
                    acts[1].append(lambda p=prv: emit_outproj(p, (6, 7)))
                if h == HPC - 1:
                    acts[2].append(lambda: emit_outproj(h, (4, 5)))
                    acts[3].append(lambda: emit_outproj(h, (6, 7)))
                if nxt < HPC:
                    acts[1].append(lambda n=nxt: emit_proj_qk(n))
                    acts[2].append(lambda n=nxt: emit_proj_v(n))
                    acts[3].append(lambda n=nxt: emit_vp(n))
                if debug and h == 0:
                    nc.gpsimd.dma_start(dqT[:], st[0]["qT"][:])
                    nc.gpsimd.dma_start(dkT[:], st[0]["kT"][:])
                    nc.gpsimd.dma_start(dvT[:], st[0]["vT"][:])
                    for i in range(NT):
                        nc.gpsimd.dma_start(dvp[:, i, :], st[0]["vp"][i][:, 0:DH])
                emit_pass(h, (0, 1), {c: acts[c] for c in (0, 1)})
                emit_outproj(h, (0, 1, 2, 3))
                emit_pass(h, (2, 3), {c: acts[c] for c in (2, 3)})
                if debug and h == 0:
                    nc.gpsimd.dma_start(dzT[:], st[0]["zT"][:])
                    nc.gpsimd.dma_start(drc[:], st[0]["rc"][:])

    nc.compile()
    return nc


_CACHED = None


def _program(debug=False):
    global _CACHED
    if _CACHED is None:
        _CACHED = build_program(debug)
    return _CACHED


def _make_in_maps(inputs):
    xq_f = np.asarray(inputs["normalized_resid_pre_q"], dtype=np.float32)
    xk_f = np.asarray(inputs["normalized_resid_pre_k"], dtype=np.float32)
    xv_f = np.asarray(inputs["normalized_resid_pre_v"], dtype=np.float32)
    WQ = np.asarray(inputs["W_Q"], dtype=np.float32) * WSC
    WK = np.asarray(inputs["W_K"], dtype=np.float32) * WSC
    WV = np.asarray(inputs["W_V"], dtype=np.float32)
    WO = np.asarray(inputs["W_O"], dtype=np.float32)
    bQ = np.asarray(inputs["b_Q"], dtype=np.float32) * WSC
    bK = np.asarray(inputs["b_K"], dtype=np.float32) * WSC
    bV = np.asarray(inputs["b_V"], dtype=np.float32)
    bO = np.asarray(inputs["b_O"], dtype=np.float32)

    def interleave_x(x):  # [DM, S] -> [2, 128, 3, 2, 1024] (s-halves split)
        y = x.reshape(3, 2, 128, 2, 1024)
        return np.ascontiguousarray(y.transpose(3, 2, 0, 1, 4))

    def interleave_w(w):  # [DM, DH] -> [128, 3, 2, DH]
        return np.ascontiguousarray(
            w.reshape(3, 2, 128, DH).transpose(2, 0, 1, 3))

    identh = np.eye(128, dtype=np.float16)
    masku = ((np.arange(128)[:, None] > np.arange(128)[None, :])
             .astype(np.float16) * np.float16(NEG))

    bq2 = np.zeros((H, 128, 1), np.float32)
    bq2[:, 0:DH, 0] = bQ
    bq2[:, DH:128, 0] = bQ
    bk2 = np.zeros((H, 128, 1), np.float32)
    bk2[:, 0:DH, 0] = bK
    bk2[:, DH:128, 0] = bK
    bv2 = np.zeros((H, 128, 1), np.float32)
    bv2[:, 0:DH, 0] = bV
    bv2[:, DH:128, 0] = bV

    in_maps = []
    for c in range(N_CORES):
        b = c % 2
        hg = c // 2
        hs = slice(HPC * hg, HPC * hg + HPC)
        m = {
            "xq": np.stack([interleave_x(
                xq_f[b, :, HPC * hg + j, :].T) for j in range(HPC)]
                ).astype(NP_X),
            "xk": np.stack([interleave_x(
                xk_f[b, :, HPC * hg + j, :].T) for j in range(HPC)]
                ).astype(NP_X),
            "xv": np.ascontiguousarray(
                xv_f[b, :, hs, :].transpose(1, 2, 0)
                .reshape(HPC, MT, 128, 2, 1024).transpose(0, 3, 2, 1, 4)
                ).astype(np.float16),
            "wp8": np.concatenate(sum((
                [interleave_w(WQ[HPC * hg + j]).reshape(128, 384),
                 interleave_w(WK[HPC * hg + j]).reshape(128, 384)]
                for j in range(HPC)), []), axis=1).astype(NP_X),
            "wp16": np.concatenate(sum((
                [WV[HPC * hg + j].reshape(MT, 128, DH)
                 .transpose(1, 0, 2).reshape(128, 384),
                 np.concatenate([WO[HPC * hg + j], WO[HPC * hg + j]],
                                axis=0)]
                for j in range(HPC)), []), axis=1).astype(np.float16),
            "bp": np.concatenate(sum((
                [bq2[HPC * hg + j], bk2[HPC * hg + j], bv2[HPC * hg + j]]
                for j in range(HPC)), []), axis=1).astype(np.float32),
            "identh": identh,
            "masku": masku,
        }
        in_maps.append(m)
    return in_maps


def run(inputs, trace=False, debug=False, **kw):
    nc = _program(debug)
    in_maps = _make_in_maps(inputs)
    res = run_bass_kernel_spmd(nc, in_maps, core_ids=list(range(N_CORES)),
                               trace=trace, **kw)
    full = np.zeros((B, S, H, DM), np.float32)
    for c in range(N_CORES):
        b = c % 2
        hg = c // 2
        o = res.results[c]["out"]
        for j in range(HPC):
            full[b, :, HPC * hg + j, :] = o[j]
    bO = np.asarray(inputs["b_O"], dtype=np.float32)
    if np.any(bO):
        full += bO / H
    return full, res


def kernel(**inputs):
    full, _ = run(inputs)
    return full
